# revision 1
# baseline (speedup 1.0000x reference)
"""Attention-pooling kernel (AttLayer) for Trainium2, 8 NeuronCores.

Math (per batch b):
    uit  = tanh(x @ W + b)          # [T, A]
    ait  = exp(uit @ u)             # [T]
    out  = (sum_t ait[t] * x[t,:]) / (sum_t ait[t] + EPS)   # [D]

Single pass over x: accumulate numerator and denominator together.

Device data layout (per core, pure data-parallel over batch; host casts to
bf16 and pre-transposes so x streams with d on partitions at full DMA rate):
    xt   [2, 128, BL*T] bf16  -- host-pre-transposed x: (d_chunk, d_in_chunk, b*t)
    w    [2, 128, A]    bf16  -- W split into two 128-row d-chunks
    bb   [A, 1]         f32   -- bias, per-partition for ACT
    urep [A, 128]       bf16  -- u tiled to 128 columns (logit-broadcast trick)
Outputs:
    num  [128, 2*BL]    f32   -- numerator, col = b*2 + c
    den  [1, NQ*BL]     f32   -- per-quarter exp-sum partials, col = b*NQ + qi

Pipeline per (b, t-quarter), software-pipelined by one quarter:
    PE : psum_uitT[A, TQ]   = W0^T @ xt0 + W1^T @ xt1        (contract d)
    ACT: uitT_sb            = tanh(psum_uitT + bb)           (bias per-partition)
    PE : psum_logit[128,TQ] = urep^T @ uitT_sb               (contract a; all
                              128 output rows identical = partition-broadcast)
    ACT: e_sb               = exp(psum_logit), accum_out -> den partial
    DVE: custom TENSOR_TENSOR_REDUCE(xt_c * e_sb) -> num[:, b*2+c], seeded
         with the previous quarter's accumulator (fused multiply+reduce)
Final division num/(den+EPS) happens on host during unsharding.
"""

import sys
import types

sys.path.insert(0, "/opt/trn_rl_repo")

# bass_utils' trace path imports antenv.axon_hooks, which not every image
# ships; register a no-op fallback so trace=True degrades instead of crashing.
try:
    import antenv.axon_hooks  # noqa: F401
except ImportError:
    try:
        import antenv

        _hooks = types.ModuleType("antenv.axon_hooks")
        _hooks._HOOK = None

        def _set_hook(hook):
            _hooks._HOOK = hook

        def _get_hook():
            return _hooks._HOOK

        _hooks.set_axon_ntff_profile_hook = _set_hook
        _hooks.get_axon_ntff_profile_hook = _get_hook
        sys.modules["antenv.axon_hooks"] = _hooks
        antenv.axon_hooks = _hooks
    except ImportError:
        pass

import numpy as np
import ml_dtypes

import concourse.bacc as bacc
import concourse.tile as tile
from concourse import mybir
from concourse import bass_utils
from concourse.dve_ops import TENSOR_TENSOR_REDUCE

B, T, D, A = 64, 4096, 256, 50
NCORES = 8
BL = B // NCORES  # batches per core
EPS = 1e-7
P = 128
NCH = D // P  # 2 d-chunks


def build_attpool(nc, aps, BL, T):
    """Emit the tile program. aps: dict name->AP for dram tensors."""
    TH = T // 2   # half (TTR granularity)
    TQ = T // 4   # quarter (ACT/psum granularity)
    NQ = 4
    xt, w, bb, urep = aps["xt"], aps["w"], aps["bb"], aps["urep"]
    num, den = aps["num"], aps["den"]
    f32 = mybir.dt.float32
    bf16 = mybir.dt.bfloat16

    with tile.TileContext(nc) as tc:
        with (
            tc.tile_pool(name="singles", bufs=1) as singles,
            tc.tile_pool(name="x0", bufs=8) as x0_pool,
            tc.tile_pool(name="x1", bufs=8) as x1_pool,
            tc.tile_pool(name="uitT", bufs=3) as uitT_pool,
            tc.tile_pool(name="e", bufs=4) as e_pool,
            tc.tile_pool(name="scratch", bufs=3) as scratch_pool,
            tc.tile_pool(name="ps_uitT", bufs=2, space="PSUM") as ps_uitT_pool,
            tc.tile_pool(name="ps_logit", bufs=2, space="PSUM") as ps_logit_pool,
        ):
            # constants
            w_sb = [
                singles.tile([P, A], bf16, tag=f"w{c}", name=f"w_sb{c}")
                for c in range(NCH)
            ]
            for c in range(NCH):
                nc.sync.dma_start(out=w_sb[c][:, :], in_=w[c, :, :])
            bb_sb = singles.tile([A, 1], f32)
            urep_sb = singles.tile([A, P], bf16)
            num_sb = singles.tile([P, NCH * BL], f32)
            den_sb = singles.tile([P, NQ * BL], f32)

            # flatten all quarters; software-pipeline by one stage so PE's
            # step1(i+1) is emitted before step2(i): keeps PE/ACT streaming
            # instead of ping-ponging on the s1->tanh->s2->exp chain.
            quarters = [
                (b, h, q) for b in range(BL) for h in range(2) for q in range(2)
            ]

            xt_tiles = {}   # (b, h) -> [tile_c0, tile_c1], each [P, TH]
            e_tiles = {}    # (b, h) -> e tile [P, TH]

            def load_half(b, h):
                xt_t = [None, None]
                for c, pool in ((0, x0_pool), (1, x1_pool)):
                    xt_t[c] = pool.tile(
                        [P, TH], bf16, tag=f"xt{c}", name=f"xt_t{c}"
                    )
                    nc.sync.dma_start(
                        out=xt_t[c][:, :],
                        in_=xt[c, :, b * T + h * TH : b * T + (h + 1) * TH],
                    )
                xt_tiles[(b, h)] = xt_t

            def stage1(b, h, q):
                if (b, h) not in xt_tiles:
                    load_half(b, h)
                xt_t = xt_tiles[(b, h)]
                off = q * TQ
                ps_uitT = ps_uitT_pool.tile([A, TQ], f32, tag="psu")
                # W0 for both 512-blocks, then W1: adjacent same-weight MMs
                for c in range(NCH):
                    for s in range(0, TQ, 512):
                        sw = min(512, TQ - s)
                        nc.tensor.matmul(
                            ps_uitT[:, s : s + sw],
                            lhsT=w_sb[c][:, :],
                            rhs=xt_t[c][:, off + s : off + s + sw],
                            start=(c == 0),
                            stop=(c == NCH - 1),
                        )
                return ps_uitT

            def stage2(state):
                (b, h, q), ps_uitT = state
                qi = h * 2 + q
                uitT_sb = uitT_pool.tile([A, TQ], bf16, tag="uitT")
                nc.scalar.activation(
                    uitT_sb[:, :], ps_uitT[:, :],
                    mybir.ActivationFunctionType.Tanh,
                    bias=bb_sb[:, :],
                )
                ps_logit = ps_logit_pool.tile([P, TQ], f32, tag="psl")
                for s in range(0, TQ, 512):
                    sw = min(512, TQ - s)
                    nc.tensor.matmul(
                        ps_logit[:, s : s + sw],
                        lhsT=urep_sb[:, :],
                        rhs=uitT_sb[:, s : s + sw],
                        start=True,
                        stop=True,
                    )
                if (b, h) not in e_tiles:
                    e_tiles[(b, h)] = e_pool.tile([P, TH], f32, tag="e", name="e_sb")
                e_sb = e_tiles[(b, h)]
                nc.scalar.activation(
                    e_sb[:, q * TQ : (q + 1) * TQ], ps_logit[:, :],
                    mybir.ActivationFunctionType.Exp,
                    accum_out=den_sb[:, b * NQ + qi : b * NQ + qi + 1],
                )
                # fused multiply+reduce on DVE. Batches 0..BL-2 use one TTR
                # per half (lower DVE fixed cost); the final batch goes per
                # quarter so the last TTR starts right after the last exp and
                # the kernel tail stays short.
                xt_t = xt_tiles[(b, h)]
                if b == BL - 1:
                    for c in range(NCH):
                        col = b * NCH + c
                        scr = scratch_pool.tile([P, TQ], bf16, tag="scr", name="scr_q")
                        nc.vector._custom_dve(
                            TENSOR_TENSOR_REDUCE,
                            out=scr[:, :],
                            in0=xt_t[c][:, q * TQ : (q + 1) * TQ],
                            in1=e_sb[:, q * TQ : (q + 1) * TQ],
                            s0=0.0 if qi == 0 else num_sb[:, col : col + 1],
                            s1=1.0,
                            accum_out=num_sb[:, col : col + 1],
                        )
                elif q == 1:
                    for c in range(NCH):
                        col = b * NCH + c
                        scr = scratch_pool.tile([P, TH], bf16, tag="scr", name="scr_h")
                        nc.vector._custom_dve(
                            TENSOR_TENSOR_REDUCE,
                            out=scr[:, :],
                            in0=xt_t[c][:, :],
                            in1=e_sb[:, :],
                            s0=0.0 if h == 0 else num_sb[:, col : col + 1],
                            s1=1.0,
                            accum_out=num_sb[:, col : col + 1],
                        )
                if q == 1:
                    xt_tiles.pop((b, h))
                    del e_tiles[(b, h)]
                if qi == 3 and b == BL - 2:
                    # drain all finished batches' numerators early so only
                    # the last batch's 2 columns remain for the tail DMA
                    nc.sync.dma_start(
                        out=num[:, : (BL - 1) * NCH],
                        in_=num_sb[:, : (BL - 1) * NCH],
                    )

            pend = None
            first = True
            for qd in quarters:
                ps = stage1(*qd)
                if first:
                    # bias/urep consts load after the first x tiles so the
                    # critical first matmul's data is in front of the queue
                    nc.sync.dma_start(out=bb_sb[:, :], in_=bb[:, :])
                    nc.sync.dma_start(out=urep_sb[:, :], in_=urep[:, :])
                    first = False
                if pend is not None:
                    stage2(pend)
                pend = (qd, ps)
            stage2(pend)
            nc.sync.dma_start(
                out=num[:, (BL - 1) * NCH :], in_=num_sb[:, (BL - 1) * NCH :]
            )
            nc.sync.dma_start(out=den[:, :], in_=den_sb[0:1, :])
    return nc


def _declare(nc, BL, T):
    f32 = mybir.dt.float32
    bf16 = mybir.dt.bfloat16
    NQ = 4
    aps = {
        "xt": nc.dram_tensor("xt", (NCH, P, BL * T), bf16, kind="ExternalInput").ap(),
        "w": nc.dram_tensor("w", (NCH, P, A), bf16, kind="ExternalInput").ap(),
        "bb": nc.dram_tensor("bb", (A, 1), f32, kind="ExternalInput").ap(),
        "urep": nc.dram_tensor("urep", (A, P), bf16, kind="ExternalInput").ap(),
        "num": nc.dram_tensor("num", (P, NCH * BL), f32, kind="ExternalOutput").ap(),
        "den": nc.dram_tensor("den", (1, NQ * BL), f32, kind="ExternalOutput").ap(),
    }
    return aps


_CACHE = {}


def _get_nc():
    key = "nc"
    if key not in _CACHE:
        nc = bacc.Bacc(
            "TRN2", target_bir_lowering=False, debug=False,
            enable_asserts=False, num_devices=NCORES,
        )
        aps = _declare(nc, BL, T)
        build_attpool(nc, aps, BL, T)
        nc.compile()
        _CACHE[key] = nc
    return _CACHE[key]


def _host_prep(x, W, b, u):
    """Build per-core input maps from full inputs."""
    x = np.asarray(x, dtype=np.float32)
    W = np.asarray(W, dtype=np.float32)
    b = np.asarray(b, dtype=np.float32)
    u = np.asarray(u, dtype=np.float32)
    wc = np.ascontiguousarray(W.reshape(NCH, P, A)).astype(ml_dtypes.bfloat16)
    bb = np.ascontiguousarray(b.reshape(A, 1))
    urep = np.ascontiguousarray(np.tile(u.reshape(A, 1), (1, P))).astype(ml_dtypes.bfloat16)
    in_maps = []
    for core in range(NCORES):
        xc = x[core * BL : (core + 1) * BL]  # [BL, T, D]
        # -> [NCH, P, BL*T]: xt[c, dp, b*T+t] = x[b, t, c*128+dp]
        xt = np.ascontiguousarray(
            xc.reshape(BL, T, NCH, P).transpose(2, 3, 0, 1).reshape(NCH, P, BL * T)
        ).astype(ml_dtypes.bfloat16)
        in_maps.append({"xt": xt, "w": wc, "bb": bb, "urep": urep})
    return in_maps


def _unshard(results):
    out = np.empty((B, D), dtype=np.float32)
    NQ = 4
    for core in range(NCORES):
        num = results[core]["num"]          # [128, 2*BL]
        den = results[core]["den"]          # [1, NQ*BL]
        den_b = den.reshape(BL, NQ).sum(axis=1)  # [BL]
        for bl in range(BL):
            vec = np.concatenate(
                [num[:, bl * NCH + c] for c in range(NCH)]
            )  # [D]
            out[core * BL + bl] = vec / (den_b[bl] + EPS)
    return out


def kernel(x, W, b, u, _trace=False):
    nc = _get_nc()
    in_maps = _host_prep(x, W, b, u)
    res = bass_utils.run_bass_kernel_spmd(
        nc, in_maps, core_ids=list(range(NCORES)), trace=_trace,
    )
    out = _unshard(res.results)
    if _trace:
        kernel.last_result = res
    return out



# revision 17
# speedup vs baseline: 1.0549x; 1.0549x over previous
"""Attention-pooling kernel (AttLayer) for Trainium2, 8 NeuronCores.

Math (per batch b):
    uit  = tanh(x @ W + b)          # [T, A]
    e    = exp(uit @ u)             # [T]
    out  = (sum_t e[t] * x[t,:]) / (sum_t e[t] + EPS)   # [D]

Per-core structure (pure data parallel over batch, BL=8 batches/core),
processing halves of T (TH=2048) so every engine streams concurrently:

    PE  : ps_uitT[100, 1024] = W^T @ x-half   (both 1024-quarters of the
          half packed on the partition axis: q0 -> rows 0-49, q1 -> 50-99;
          matmuls grouped per stationary: LDW w0 x4MM, LDW w1 x4MM)
    ACT : uitT = tanh(ps_uitT + bias2) as ONE [100, 1024] instr
    PE  : ps_logit[128, 1024] = urep^T @ uitT[rows]  per quarter
          (128 identical rows = partition-broadcast of the logit)
    ACT : e[:, quarter] = exp(ps_logit)  -> e_sb [128, 2048] bf16
    DVE : scalar_tensor_tensor(x*e, accum_out) per chunk [128, 2048]
          (the only engine that can fuse multiply+free-axis reduce;
          Pool rejects TensorScalarPtr at the v3 ISA level)

num partials land in num_parts[128, 32] (col = (b*2+c)*2 + h), one row
of e per half is DMA'd out; host sums partials, computes den = sum(e)
and the final division.  DVE at 1 col/0.96GHz-cycle over 65.5k columns
(~70us) is the critical engine; DMA (~48.5us for the 16.8MB/core bf16
x stream), PE (~43us) and ACT (~43us) hide under it.
"""

import sys
import types

sys.path.insert(0, "/opt/trn_rl_repo")

# bass_utils' trace path imports antenv.axon_hooks, which not every image
# ships; register a no-op fallback so trace=True degrades instead of crashing.
try:
    import antenv.axon_hooks  # noqa: F401
except ImportError:
    try:
        import antenv

        _hooks = types.ModuleType("antenv.axon_hooks")
        _hooks._HOOK = None

        def _set_hook(hook):
            _hooks._HOOK = hook

        def _get_hook():
            return _hooks._HOOK

        _hooks.set_axon_ntff_profile_hook = _set_hook
        _hooks.get_axon_ntff_profile_hook = _get_hook
        sys.modules["antenv.axon_hooks"] = _hooks
        antenv.axon_hooks = _hooks
    except ImportError:
        pass

import numpy as np
import ml_dtypes

import concourse.bacc as bacc
import concourse.tile as tile
from concourse import mybir
from concourse import bass_utils

B, T, D, A = 64, 4096, 256, 50
NCORES = 8
BL = B // NCORES  # batches per core
EPS = 1e-7
P = 128
NCH = D // P      # 2 d-chunks
TH = T // 2       # 2048: half, the pipeline granule
TQ = T // 4       # 1024: quarter (psum granule)
NH = 2 * BL       # 16 halves per core
NPART = 2         # partial slots per (b, c): one per half h


def build_attpool(nc, aps):
    xt, w, bb2, urep = aps["xt"], aps["w"], aps["bb2"], aps["urep"]
    nump, eout = aps["nump"], aps["eout"]
    f32 = mybir.dt.float32
    bf16 = mybir.dt.bfloat16
    LOOKAHEAD = 6

    with tile.TileContext(nc) as tc:
        with (
            tc.tile_pool(name="singles", bufs=1) as singles,
            tc.tile_pool(name="x0", bufs=10) as x0_pool,
            tc.tile_pool(name="x1", bufs=10) as x1_pool,
            tc.tile_pool(name="uitT", bufs=2) as uitT_pool,
            tc.tile_pool(name="e", bufs=3) as e_pool,
            tc.tile_pool(name="scrd", bufs=2) as scrd_pool,
            tc.tile_pool(name="ps_uitT", bufs=2, space="PSUM") as ps_uitT_pool,
            tc.tile_pool(name="ps_logit", bufs=2, space="PSUM") as ps_logit_pool,
        ):
            # constants + persistent outputs
            w_sb = [
                singles.tile([P, A], bf16, tag=f"w{c}", name=f"w_sb{c}")
                for c in range(NCH)
            ]
            for c in range(NCH):
                nc.sync.dma_start(out=w_sb[c][:, :], in_=w[c, :, :])
            bb2_sb = singles.tile([P, 1], f32)
            urep_sb = singles.tile([P, P], bf16)
            nc.sync.dma_start(out=bb2_sb[:, :], in_=bb2[:, :])
            nc.sync.dma_start(out=urep_sb[:, :], in_=urep[:, :])
            num_parts = singles.tile([P, NCH * BL * NPART], f32)

            halves = [(b, h) for b in range(BL) for h in range(2)]
            xt_tiles = {}

            def load_half(i):
                b, h = halves[i]
                t0 = b * T + h * TH
                xt_t = []
                for c, pool in ((0, x0_pool), (1, x1_pool)):
                    tl = pool.tile([P, TH], bf16, tag=f"xt{c}", name=f"xt{c}_{i}")
                    nc.sync.dma_start(out=tl[:, :], in_=xt[c, :, t0 : t0 + TH])
                    xt_t.append(tl)
                xt_tiles[i] = xt_t

            def stage1(i):
                """mm1 for half i -> ps_uitT [128, 1024]; quarter q on rows
                64*q..64*q+49 (PE out base partition must be 0/32/64)."""
                xt_t = xt_tiles[i]
                ps = ps_uitT_pool.tile([P, TQ], f32, tag="psu")
                for c in range(NCH):  # stationary-major: 1 LDW per chunk
                    for q in range(2):
                        for s in (0, 512):
                            nc.tensor.matmul(
                                ps[64 * q : 64 * q + A, s : s + 512],
                                lhsT=w_sb[c][:, :],
                                rhs=xt_t[c][:, q * TQ + s : q * TQ + s + 512],
                                start=(c == 0),
                                stop=(c == NCH - 1),
                            )
                return ps

            def stage2(i, ps_uitT):
                """tanh, mm2+exp per quarter, numerator STTs for half i."""
                b, h = halves[i]
                uitT_sb = uitT_pool.tile([P, TQ], bf16, tag="uitT")
                nc.scalar.activation(
                    uitT_sb[0 : 64 + A, :], ps_uitT[0 : 64 + A, :],
                    mybir.ActivationFunctionType.Tanh,
                    bias=bb2_sb[0 : 64 + A, :],
                )
                e_sb = e_pool.tile([P, TH], f32 if False else bf16, tag="e")
                for q in range(2):
                    ps_logit = ps_logit_pool.tile([P, TQ], f32, tag="psl")
                    for s in (0, 512):
                        nc.tensor.matmul(
                            ps_logit[:, s : s + 512],
                            lhsT=urep_sb[64 * q : 64 * q + A, :],
                            rhs=uitT_sb[64 * q : 64 * q + A, s : s + 512],
                            start=True,
                            stop=True,
                        )
                    nc.scalar.activation(
                        e_sb[:, q * TQ : (q + 1) * TQ], ps_logit[:, :],
                        mybir.ActivationFunctionType.Exp,
                    )
                # numerator: accum_out = sum_t x*e per partition, per chunk
                xt_t = xt_tiles[i]
                for c in range(NCH):
                    slot = (b * NCH + c) * NPART + h
                    scr = scrd_pool.tile([P, TH], bf16, tag="scrd")
                    nc.vector.scalar_tensor_tensor(
                        out=scr[:, :],
                        in0=xt_t[c][:, :], scalar=1.0, in1=e_sb[:, :],
                        op0=mybir.AluOpType.mult, op1=mybir.AluOpType.mult,
                        accum_out=num_parts[:, slot : slot + 1],
                    )
                # one (identical) row of e out for the host-side denominator
                nc.sync.dma_start(out=eout[i : i + 1, :], in_=e_sb[0:1, :])
                del xt_tiles[i]

            for i in range(min(LOOKAHEAD, NH)):
                load_half(i)
            pend = None
            for i in range(NH):
                ps = stage1(i)
                if i + LOOKAHEAD < NH:
                    load_half(i + LOOKAHEAD)
                if pend is not None:
                    stage2(*pend)
                pend = (i, ps)
            stage2(*pend)
            nc.sync.dma_start(out=nump[:, :], in_=num_parts[:, :])
    return nc


def _declare(nc):
    f32 = mybir.dt.float32
    bf16 = mybir.dt.bfloat16
    aps = {
        "xt": nc.dram_tensor("xt", (NCH, P, BL * T), bf16, kind="ExternalInput").ap(),
        "w": nc.dram_tensor("w", (NCH, P, A), bf16, kind="ExternalInput").ap(),
        "bb2": nc.dram_tensor("bb2", (P, 1), f32, kind="ExternalInput").ap(),
        "urep": nc.dram_tensor("urep", (P, P), bf16, kind="ExternalInput").ap(),
        "nump": nc.dram_tensor(
            "nump", (P, NCH * BL * NPART), f32, kind="ExternalOutput"
        ).ap(),
        "eout": nc.dram_tensor("eout", (NH, TH), bf16, kind="ExternalOutput").ap(),
    }
    return aps


_CACHE = {}


def _get_nc():
    key = "nc"
    if key not in _CACHE:
        nc = bacc.Bacc(
            "TRN2", target_bir_lowering=False, debug=False,
            enable_asserts=False, num_devices=NCORES,
        )
        aps = _declare(nc)
        build_attpool(nc, aps)
        nc.compile()
        _CACHE[key] = nc
    return _CACHE[key]


def _host_prep(x, W, b, u):
    """Build per-core input maps from full inputs (layout/dtype prep only)."""
    x = np.asarray(x, dtype=np.float32)
    W = np.asarray(W, dtype=np.float32)
    b = np.asarray(b, dtype=np.float32)
    u = np.asarray(u, dtype=np.float32)
    wc = np.ascontiguousarray(W.reshape(NCH, P, A)).astype(ml_dtypes.bfloat16)
    bb2 = np.zeros((P, 1), dtype=np.float32)
    bb2[0:A, 0] = b
    bb2[64 : 64 + A, 0] = b
    urep = np.zeros((P, P), dtype=np.float32)
    urep[0:A, :] = u.reshape(A, 1)
    urep[64 : 64 + A, :] = u.reshape(A, 1)
    urep = np.ascontiguousarray(urep).astype(ml_dtypes.bfloat16)
    in_maps = []
    for core in range(NCORES):
        xc = x[core * BL : (core + 1) * BL]  # [BL, T, D]
        # -> [NCH, P, BL*T]: xt[c, dp, b*T+t] = x[b, t, c*128+dp]
        xt = np.ascontiguousarray(
            xc.reshape(BL, T, NCH, P).transpose(2, 3, 0, 1).reshape(NCH, P, BL * T)
        ).astype(ml_dtypes.bfloat16)
        in_maps.append({"xt": xt, "w": wc, "bb2": bb2, "urep": urep})
    return in_maps


def _unshard(results):
    out = np.empty((B, D), dtype=np.float32)
    for core in range(NCORES):
        nump = results[core]["nump"]              # [128, 2*BL*NPART] f32
        eout = np.asarray(results[core]["eout"])  # [NH, TH] bf16
        parts = nump.reshape(P, BL, NCH, NPART).sum(axis=3)  # [128, BL, NCH]
        den = eout.astype(np.float32).reshape(BL, 2 * TH).sum(axis=1)  # [BL]
        for bl in range(BL):
            vec = np.concatenate([parts[:, bl, 0], parts[:, bl, 1]])  # [D]
            out[core * BL + bl] = vec / (den[bl] + EPS)
    return out


def kernel(x, W, b, u, _trace=False):
    nc = _get_nc()
    in_maps = _host_prep(x, W, b, u)
    res = bass_utils.run_bass_kernel_spmd(
        nc, in_maps, core_ids=list(range(NCORES)), trace=_trace,
    )
    out = _unshard(res.results)
    if _trace:
        kernel.last_result = res
    return out
